# revision 2
# baseline (speedup 1.0000x reference)
"""GNN message-passing MLP on 8 Trainium2 NeuronCores — dense edge-parallel.

Computes, for each of 2 "mc" embedding tables x (shape [N, 128]) and each of
500K edges (src, dst):
    y = relu(x[src] @ W1a + x[dst] @ W1b + b1) @ W2 + b2        # [2, E, 128]

Distribution: edge-parallel across 8 cores, no collectives. The host stages
the per-edge endpoint features (the "gather") into dense feature-major fp16
arrays, so the device runs a pure streaming MLP at the memory/PE roofline:

- Per core input x: [4, 128, EC] fp16, plane k = 2*mc + (0=src, 1=dst),
  laid out [feature, edge] so tiles load straight into matmul rhs layout.
- L1 runs weight-stationary (h^T = W1c^T x^T in PSUM, K=256 split in two
  128-chunks over src/dst planes), bias+relu fused on the scalar engine.
- L2 is W2-stationary (y^T = W2^T h^T), b2 added during the PSUM->SBUF
  fp16 cast on the vector engine.
- y returns as [2, 128, EC] fp16 (feature-major); the host transposes back
  to [2, E, 128] fp32.
"""

import os
import sys

import numpy as np

for _p in ("/opt/trn_rl_repo", "/root/.axon_site/_ro/trn_rl_repo"):
    if os.path.isdir(_p) and _p not in sys.path:
        sys.path.insert(0, _p)

import concourse.bass as bass  # noqa: F401
import concourse.mybir as mybir
import concourse.tile as tile
from concourse import bacc
from concourse.bass_utils import run_bass_kernel_spmd

# Problem constants (hardcoded per harness contract).
N_NODES = 100000
E_TOTAL = 500000
D = 128          # input feature dim
H = 256          # hidden dim
O = 128          # output dim
MC = 2           # number of embedding tables
CORES = 8
P = 128

E_CORE = E_TOTAL // CORES    # real edges per core (62500)
TILE = 2048                  # edges per DMA tile
NT = (E_CORE + TILE - 1) // TILE  # 31
EC = NT * TILE               # padded edges per core (63488)
SUB = 512                    # edges per matmul (PSUM bank width)
GRP = 1024                   # edges per PSUM tile / elementwise op

_CACHE = {}
_last_in_maps = None


def _build(repeats=1):
    f16 = mybir.dt.float16
    f32 = mybir.dt.float32

    nc = bacc.Bacc("TRN2", target_bir_lowering=False, num_devices=CORES)
    x = nc.declare_dram_parameter("x", [2 * MC, P, EC], f16, isOutput=False)
    w1 = nc.declare_dram_parameter("w1", [P, 2, H // P, P], f16, isOutput=False)
    w2 = nc.declare_dram_parameter("w2", [P, H // P, O], f16, isOutput=False)
    b1 = nc.declare_dram_parameter("b1", [P, H // P], f32, isOutput=False)
    b2 = nc.declare_dram_parameter("b2", [P, 1], f32, isOutput=False)
    y = nc.declare_dram_parameter("y", [MC, P, EC], f16, isOutput=True)

    relu = mybir.ActivationFunctionType.Relu
    add = mybir.AluOpType.add

    with tile.TileContext(nc) as tc:
        with (
            tc.tile_pool(name="const", bufs=1) as cpool,
            tc.tile_pool(name="xg", bufs=2) as xpool,
            tc.tile_pool(name="ht", bufs=2) as htpool,
            tc.tile_pool(name="yo", bufs=2) as yopool,
            tc.tile_pool(name="ph", bufs=2, space="PSUM") as phpool,
            tc.tile_pool(name="py", bufs=2, space="PSUM") as pypool,
        ):
            w1_sb = cpool.tile([P, 2, H // P, P], f16)   # [feat, a/b, chunk, h]
            nc.sync.dma_start(w1_sb[:], w1[:])
            w2_sb = cpool.tile([P, H // P, O], f16)      # [h_in_chunk, chunk, o]
            nc.sync.dma_start(w2_sb[:], w2[:])
            b1_sb = cpool.tile([P, H // P], f32)
            nc.sync.dma_start(b1_sb[:], b1[:])
            b2_sb = cpool.tile([P, 1], f32)
            nc.sync.dma_start(b2_sb[:], b2[:])

            for _rep in range(repeats):
                for t in range(NT):
                    e0 = t * TILE
                    xt = []
                    for k in range(2 * MC):
                        xk = xpool.tile([P, TILE], f16, tag=f"x{k}")
                        nc.sync.dma_start(xk[:], x[k, :, e0:e0 + TILE])
                        xt.append(xk)

                    for mc in range(MC):
                        yo = yopool.tile([P, TILE], f16, tag=f"yo{mc}")
                        for g in range(TILE // GRP):
                            hts = []
                            for c in range(H // P):
                                ph = phpool.tile([P, GRP], f32, tag="ph")
                                for i in range(GRP // SUB):
                                    o_ = g * GRP + i * SUB
                                    nc.tensor.matmul(
                                        ph[:, i * SUB:(i + 1) * SUB],
                                        lhsT=w1_sb[:, 0, c, :],
                                        rhs=xt[2 * mc][:, o_:o_ + SUB],
                                        start=True, stop=False,
                                    )
                                for i in range(GRP // SUB):
                                    o_ = g * GRP + i * SUB
                                    nc.tensor.matmul(
                                        ph[:, i * SUB:(i + 1) * SUB],
                                        lhsT=w1_sb[:, 1, c, :],
                                        rhs=xt[2 * mc + 1][:, o_:o_ + SUB],
                                        start=False, stop=True,
                                    )
                                ht = htpool.tile([P, GRP], f16, tag=f"ht{c}")
                                nc.scalar.activation(
                                    ht[:], ph[:], relu,
                                    bias=b1_sb[:, c:c + 1],
                                )
                                hts.append(ht)

                            py = pypool.tile([P, GRP], f32, tag="py")
                            for c in range(H // P):
                                for i in range(GRP // SUB):
                                    nc.tensor.matmul(
                                        py[:, i * SUB:(i + 1) * SUB],
                                        lhsT=w2_sb[:, c, :],
                                        rhs=hts[c][:, i * SUB:(i + 1) * SUB],
                                        start=(c == 0),
                                        stop=(c == H // P - 1),
                                    )
                            nc.vector.tensor_scalar_add(
                                yo[:, g * GRP:(g + 1) * GRP],
                                py[:],
                                b2_sb[:, 0:1],
                            )
                        nc.sync.dma_start(y[mc, :, e0:e0 + TILE], yo[:])

    nc.compile()
    return nc


def _get_program(repeats=1):
    if repeats not in _CACHE:
        _CACHE[repeats] = _build(repeats)
    return _CACHE[repeats]


def kernel(edge_index, mc_embeddings, W1, b1, W2, b2):
    nc = _get_program(1)

    edge_index = np.asarray(edge_index)
    mc_embeddings = np.asarray(mc_embeddings, dtype=np.float32)
    W1 = np.asarray(W1, dtype=np.float32)
    b1 = np.asarray(b1, dtype=np.float32)
    W2 = np.asarray(W2, dtype=np.float32)
    b2 = np.asarray(b2, dtype=np.float32)

    tab16 = mc_embeddings.astype(np.float16)            # [MC, N, D]
    w1_in = np.ascontiguousarray(
        W1.reshape(2, P, H // P, P).transpose(1, 0, 2, 3)
    ).astype(np.float16)
    w2_in = np.ascontiguousarray(
        W2.reshape(H // P, P, O).transpose(1, 0, 2)
    ).astype(np.float16)
    b1_in = np.ascontiguousarray(b1.reshape(H // P, P).T).astype(np.float32)
    b2_in = np.ascontiguousarray(b2[:, None]).astype(np.float32)

    idx64 = edge_index.astype(np.int64)
    in_maps = []
    for c in range(CORES):
        lo = c * E_CORE
        src = idx64[0, lo:lo + E_CORE]
        dst = idx64[1, lo:lo + E_CORE]
        xc = np.zeros((2 * MC, P, EC), dtype=np.float16)
        for mc in range(MC):
            xc[2 * mc, :, :E_CORE] = tab16[mc, src, :].T
            xc[2 * mc + 1, :, :E_CORE] = tab16[mc, dst, :].T
        in_maps.append({
            "x": xc,
            "w1": w1_in,
            "w2": w2_in,
            "b1": b1_in,
            "b2": b2_in,
        })

    global _last_in_maps
    _last_in_maps = in_maps
    res = run_bass_kernel_spmd(nc, in_maps, list(range(CORES)))

    out = np.empty((MC, E_TOTAL, O), dtype=np.float32)
    for c in range(CORES):
        lo = c * E_CORE
        yv = res.results[c]["y"]                     # [MC, P, EC] fp16
        out[:, lo:lo + E_CORE, :] = yv[:, :, :E_CORE].transpose(0, 2, 1)
    return out


# revision 3
# speedup vs baseline: 1.0834x; 1.0834x over previous
"""GNN message-passing MLP on 8 Trainium2 NeuronCores — dense edge-parallel.

y = relu(x[src] @ W1a + x[dst] @ W1b + b1) @ W2 + b2   for 2 mc tables, 500K edges.

Distribution: edge-parallel across 8 cores, no collectives. The host stages
per-edge endpoint features (the gather) into dense feature-major planes, so
the device runs a pure streaming MLP:

- x planes ship as fp16 feature-major, loading straight into matmul rhs
  layout (DMA ~190us/pass hides fully under the ~380us compute).
- L1 weight-stationary in 2 K-chunks, relu+bias fused: mc0 tiles on the
  scalar engine, mc1 tiles on the vector engine (tensor_scalar add+max),
  so neither engine is the bottleneck.
- L2 W2-stationary with both mc accumulators live (one W2 load per chunk
  per 1024-edge group), b2 added during the PSUM->SBUF fp16 cast (mc0 on
  scalar, mc1 on vector).
- y returns [2, 128, EC] fp16 feature-major; host transposes to [2, E, 128]
  fp32.
"""

import os
import sys

import numpy as np

for _p in ("/opt/trn_rl_repo", "/root/.axon_site/_ro/trn_rl_repo"):
    if os.path.isdir(_p) and _p not in sys.path:
        sys.path.insert(0, _p)

import concourse.bass as bass  # noqa: F401
import concourse.mybir as mybir
import concourse.tile as tile
from concourse import bacc
from concourse.bass_utils import run_bass_kernel_spmd

N_NODES = 100000
E_TOTAL = 500000
D = 128
H = 256
O = 128
MC = 2
CORES = 8
P = 128

E_CORE = E_TOTAL // CORES    # 62500
TILE = 2048                  # edges per DMA tile
NT = (E_CORE + TILE - 1) // TILE  # 31
EC = NT * TILE               # 63488
SUB = 512                    # edges per matmul (PSUM bank width)
GRP = 1024                   # edges per PSUM tile / elementwise op

_CACHE = {}
_last_in_maps = None


def _build(repeats=1):
    f16 = mybir.dt.float16
    f32 = mybir.dt.float32

    nc = bacc.Bacc("TRN2", target_bir_lowering=False, num_devices=CORES)
    x = nc.declare_dram_parameter("x", [2 * MC, P, EC], f16, isOutput=False)
    w1 = nc.declare_dram_parameter("w1", [P, 2, H // P, P], f16, isOutput=False)
    w2 = nc.declare_dram_parameter("w2", [P, H // P, O], f16, isOutput=False)
    b1 = nc.declare_dram_parameter("b1", [P, H // P], f32, isOutput=False)
    b2 = nc.declare_dram_parameter("b2", [P, 1], f32, isOutput=False)
    y = nc.declare_dram_parameter("y", [MC, P, EC], f16, isOutput=True)

    relu = mybir.ActivationFunctionType.Relu
    ident = mybir.ActivationFunctionType.Identity
    add = mybir.AluOpType.add
    amax = mybir.AluOpType.max

    with tile.TileContext(nc) as tc:
        with (
            tc.tile_pool(name="const", bufs=1) as cpool,
            tc.tile_pool(name="xg", bufs=2) as xpool,
            tc.tile_pool(name="ht", bufs=2) as htpool,
            tc.tile_pool(name="yo", bufs=2) as yopool,
            tc.tile_pool(name="ph", bufs=2, space="PSUM") as phpool,
            tc.tile_pool(name="py", bufs=2, space="PSUM") as pypool,
        ):
            w1_sb = cpool.tile([P, 2, H // P, P], f16)   # [feat, a/b, chunk, h]
            nc.sync.dma_start(w1_sb[:], w1[:])
            w2_sb = cpool.tile([P, H // P, O], f16)      # [h_in_chunk, chunk, o]
            nc.sync.dma_start(w2_sb[:], w2[:])
            b1_sb = cpool.tile([P, H // P], f32)
            nc.sync.dma_start(b1_sb[:], b1[:])
            b2_sb = cpool.tile([P, 1], f32)
            nc.sync.dma_start(b2_sb[:], b2[:])

            for _rep in range(repeats):
                for t in range(NT):
                    e0 = t * TILE
                    xt = []
                    for k in range(2 * MC):
                        xk = xpool.tile([P, TILE], f16, tag=f"x{k}")
                        (nc.sync if k % 2 == 0 else nc.gpsimd).dma_start(
                            xk[:], x[k, :, e0:e0 + TILE])
                        xt.append(xk)

                    yos = []
                    for mc in range(MC):
                        yos.append(yopool.tile([P, TILE], f16, tag=f"yo{mc}", name=f"yo{mc}"))

                    for g in range(TILE // GRP):
                        o0 = g * GRP
                        # L1: per mc, per chunk; relu mc0->ACT, mc1->DVE.
                        hts = {}
                        for mc in range(MC):
                            for c in range(H // P):
                                ph = phpool.tile([P, GRP], f32, tag="ph")
                                for ab in range(2):
                                    for i in range(GRP // SUB):
                                        o_ = o0 + i * SUB
                                        nc.tensor.matmul(
                                            ph[:, i * SUB:(i + 1) * SUB],
                                            lhsT=w1_sb[:, ab, c, :],
                                            rhs=xt[2 * mc + ab][:, o_:o_ + SUB],
                                            start=(ab == 0), stop=(ab == 1),
                                        )
                                ht = htpool.tile([P, GRP], f16, tag=f"ht{mc}{c}")
                                if mc == 0:
                                    nc.scalar.activation(
                                        ht[:], ph[:], relu,
                                        bias=b1_sb[:, c:c + 1],
                                    )
                                else:
                                    nc.vector.tensor_scalar(
                                        out=ht[:], in0=ph[:],
                                        scalar1=b1_sb[:, c:c + 1],
                                        scalar2=0.0,
                                        op0=add, op1=amax,
                                    )
                                hts[(mc, c)] = ht

                        # L2: both mc accumulators live; one W2 load per chunk.
                        pys = [pypool.tile([P, GRP], f32, tag="py", name=f"py{m}")
                               for m in range(MC)]
                        for c in range(H // P):
                            for mc in range(MC):
                                for i in range(GRP // SUB):
                                    nc.tensor.matmul(
                                        pys[mc][:, i * SUB:(i + 1) * SUB],
                                        lhsT=w2_sb[:, c, :],
                                        rhs=hts[(mc, c)][:, i * SUB:(i + 1) * SUB],
                                        start=(c == 0),
                                        stop=(c == H // P - 1),
                                    )
                        nc.scalar.activation(
                            yos[0][:, o0:o0 + GRP], pys[0][:], ident,
                            bias=b2_sb[:, 0:1],
                        )
                        nc.vector.tensor_scalar_add(
                            yos[1][:, o0:o0 + GRP], pys[1][:], b2_sb[:, 0:1],
                        )

                    for mc in range(MC):
                        (nc.sync if mc == 0 else nc.gpsimd).dma_start(
                            y[mc, :, e0:e0 + TILE], yos[mc][:])

    nc.compile()
    return nc


def _get_program(repeats=1):
    if repeats not in _CACHE:
        _CACHE[repeats] = _build(repeats)
    return _CACHE[repeats]


def kernel(edge_index, mc_embeddings, W1, b1, W2, b2):
    nc = _get_program(1)

    edge_index = np.asarray(edge_index)
    mc_embeddings = np.asarray(mc_embeddings, dtype=np.float32)
    W1 = np.asarray(W1, dtype=np.float32)
    b1 = np.asarray(b1, dtype=np.float32)
    W2 = np.asarray(W2, dtype=np.float32)
    b2 = np.asarray(b2, dtype=np.float32)

    tab16 = mc_embeddings.astype(np.float16)             # [MC, N, D]
    w1_in = np.ascontiguousarray(
        W1.reshape(2, P, H // P, P).transpose(1, 0, 2, 3)
    ).astype(np.float16)
    w2_in = np.ascontiguousarray(
        W2.reshape(H // P, P, O).transpose(1, 0, 2)
    ).astype(np.float16)
    b1_in = np.ascontiguousarray(b1.reshape(H // P, P).T).astype(np.float32)
    b2_in = np.ascontiguousarray(b2[:, None]).astype(np.float32)

    idx64 = edge_index.astype(np.int64)
    in_maps = []
    for c in range(CORES):
        lo = c * E_CORE
        src = idx64[0, lo:lo + E_CORE]
        dst = idx64[1, lo:lo + E_CORE]
        xc = np.zeros((2 * MC, P, EC), dtype=np.float16)
        for mc in range(MC):
            xc[2 * mc, :, :E_CORE] = tab16[mc, src, :].T
            xc[2 * mc + 1, :, :E_CORE] = tab16[mc, dst, :].T
        in_maps.append({
            "x": xc,
            "w1": w1_in,
            "w2": w2_in,
            "b1": b1_in,
            "b2": b2_in,
        })

    global _last_in_maps
    _last_in_maps = in_maps
    res = run_bass_kernel_spmd(nc, in_maps, list(range(CORES)))

    out = np.empty((MC, E_TOTAL, O), dtype=np.float32)
    for c in range(CORES):
        lo = c * E_CORE
        yv = res.results[c]["y"]                     # [MC, P, EC] fp16
        out[:, lo:lo + E_CORE, :] = yv[:, :, :E_CORE].transpose(0, 2, 1)
    return out
